# revision 23
# baseline (speedup 1.0000x reference)
"""Grouped-linear (EvolvedLoopLinear) Trainium2 Bass kernel.

Problem: out[b, j] = sum_s x[b, g*64+s] * weight[j, g*64+s] + bias[j],
with g = j % 128, for x [4096, 8192], weight [4096, 8192], bias [4096].

Strategy: data-parallel over batch across 8 cores (512 rows each), with
ALL layout work (transposes, weight gather, output interleave) done on
the host so the device kernel is a pure DMA-bound stream:

  - x is pre-transposed per core into pair-major fp16 "xt" layout
    xt[p, 512k + n] = x[n, 128k + p]  (pair k = groups 2k, 2k+1), so the
    PE's moving operand comes straight from HBM with no on-chip
    transposes at all (the f32 baseline spent 62% tensor-engine time on
    transposes and was paced by them).
  - Weights are host-gathered into block-diagonal fp16 pair blocks
    w_bd[:, 64k:64k+64] (the only live 1 MiB of the 128 MiB weight).
  - Per pair-pair t = (2t, 2t+1), two matmuls share one [128, 512] PSUM
    bank: pair 2t -> partitions 0:64, pair 2t+1 -> partitions 64:128
    (PE quadrant tile_position (0, 64), valid for 64-col stationaries).
  - Evacuation fuses the per-output bias (per-partition column in the
    transposed layout) and the fp32->fp16 cast, alternating ACT / DVE
    per tile so neither engine paces the kernel.
  - Output stays transposed+interleaved in HBM (fp16); the host undoes
    the interleave when gathering shards.

fp16 I/O halves HBM traffic vs f32 (x 8 MiB + w 1 MiB + out 4 MiB
= 13 MiB/core, ~36 us at the 358 GB/s per-core HBM roofline) and adds
only ~2e-4 relative error (inputs are unit-scale randn/kaiming).
"""
import numpy as np
from contextlib import ExitStack

import concourse.bass as bass
import concourse.tile as tile
import concourse.tile_sem_assignment as _tsa
from concourse import bacc, mybir
from concourse.bass_utils import run_bass_kernel_spmd

# HWDGE completion lanes: each DMA's *issue* waits for the completion of
# the DMA NUM_HWDGE_SEMS-back on its lane, so this is the DMA lookahead
# depth. The walrus build in this container rejects instructions carrying
# too many semaphore waits (seen at the kernel-tail drain) -- if compile
# fails with "Too many sync wait commands", lower this.
import os as _os
_tsa.NUM_HWDGE_SEMS = int(_os.environ.get("K_HWSEMS", "8"))

BATCH = 4096
IN_F = 8192
OUT_F = 4096
GROUPS = 128
STEP = 64
M_PER_G = 32          # outputs per group
N_CORES = 8
B_CORE = BATCH // N_CORES      # 512
N_PAIR = GROUPS // 2           # 64 group pairs
N_TILE = N_PAIR // 2           # 32 output tiles (pair-pairs)
# x slab sizes in pairs: 1 MiB slabs keep the PE-burst spacing under the
# 3.4 us HAM idle window (so the PE stays at 2.4 GHz instead of being
# re-throttled to 1.2 between bursts); the shrinking tail slabs keep the
# final load->matmul->store chain off the critical path.
SLAB_PAIRS = [8, 8, 8, 8, 8, 8, 8, 4, 2, 2]
assert sum(SLAB_PAIRS) == N_PAIR

f32 = mybir.dt.float32
f16 = mybir.dt.float16

WARMUP_MM = 48

_COMPILED = {}


def _build():
    if "nc" in _COMPILED:
        return _COMPILED["nc"]

    nc = bacc.Bacc("TRN2", target_bir_lowering=False, debug=False)
    xt_ap = nc.dram_tensor("xt_s", [128, N_PAIR * B_CORE], f16,
                           kind="ExternalInput").ap()
    w_ap = nc.dram_tensor("w_bd", [128, N_PAIR * 64], f16,
                          kind="ExternalInput").ap()
    b_ap = nc.dram_tensor("bias_pp", [128, N_TILE], f32,
                          kind="ExternalInput").ap()
    y_ap = nc.dram_tensor("y_s", [128, N_TILE * B_CORE], f16,
                          kind="ExternalOutput").ap()

    with tile.TileContext(nc) as tc:
        with ExitStack() as ctx:
            const_pool = ctx.enter_context(tc.tile_pool(name="const", bufs=1))
            slab_pool = ctx.enter_context(
                tc.tile_pool(name="slab", bufs=len(SLAB_PAIRS)))
            # osb depth 6: with only 3, the tail evacuations stall ~3 us
            # waiting for a 3-back store's HBM drain to free the buffer
            osb_pool = ctx.enter_context(tc.tile_pool(name="osb", bufs=6))
            ps_pool = ctx.enter_context(tc.tile_pool(name="ps", bufs=4,
                                                     space="PSUM"))

            # weights/bias down the ACT HWDGE queue so they don't delay
            # the x slab stream on the sync queue
            w_sb = const_pool.tile([128, N_PAIR * 64], f16)
            nc.scalar.dma_start(w_sb[:], w_ap[:])
            bias_sb = const_pool.tile([128, N_TILE], f32)
            nc.scalar.dma_start(bias_sb[:], b_ap[:])

            warm_sb = const_pool.tile([128, 128], f16)
            if WARMUP_MM:
                nc.vector.memset(warm_sb[:], 0.0)

            # the whole x shard fits in SBUF: issue all slab loads
            # up-front so the sync queue streams back-to-back
            slabs = []
            col = 0
            for s, npair in enumerate(SLAB_PAIRS):
                w_cols = npair * B_CORE
                sl = slab_pool.tile([128, w_cols], f16, tag="slab",
                                    name=f"slab{s}")
                nc.sync.dma_start(sl[:], xt_ap[:, col:col + w_cols])
                slabs.append((sl, col))
                col += w_cols

            # warm-up bridge on the memset tile (no DMA dependency): keeps
            # the PE activity window continuously busy from ~7 us until
            # the first slab + weights arrive (~12-14 us), so the HAM
            # clock gate opens to 2.4 GHz and the real matmul stream
            # starts warm instead of at the 1.2 GHz cold clock
            if WARMUP_MM:
                wm = ps_pool.tile([128, B_CORE], f32, tag="ps", name="warm")
                for _ in range(WARMUP_MM):
                    nc.tensor.matmul(wm[0:64, 0:128], warm_sb[:, 0:64],
                                     warm_sb[:], start=True, stop=True)

            t = 0
            for s, npair in enumerate(SLAB_PAIRS):
                sl, col = slabs[s]
                n_tiles = npair // 2
                osb = osb_pool.tile([128, n_tiles * B_CORE], f16, tag="osb")
                for tl in range(n_tiles):
                    ps = ps_pool.tile([128, B_CORE], f32, tag="ps")
                    nc.tensor.matmul(
                        ps[0:64, :],
                        w_sb[:, (2 * t) * 64:(2 * t) * 64 + 64],
                        sl[:, (2 * tl) * B_CORE:(2 * tl + 1) * B_CORE],
                        start=True, stop=True)
                    nc.tensor.matmul(
                        ps[64:128, :],
                        w_sb[:, (2 * t + 1) * 64:(2 * t + 1) * 64 + 64],
                        sl[:, (2 * tl + 1) * B_CORE:(2 * tl + 2) * B_CORE],
                        start=True, stop=True)
                    dst = osb[:, tl * B_CORE:(tl + 1) * B_CORE]
                    if t % 2 == 0:
                        nc.scalar.add(dst, ps[:], bias_sb[:, t:t + 1])
                    else:
                        nc.vector.tensor_scalar_add(dst, ps[:],
                                                    bias_sb[:, t:t + 1])
                    t += 1
                nc.scalar.dma_start(
                    y_ap[:, (col // 2):(col // 2) + n_tiles * B_CORE],
                    osb[:])

    nc.compile()
    _COMPILED["nc"] = nc
    return nc


def _host_prep(weight, bias):
    # gather: Wg[j, s] = weight[j, (j%128)*64 + s] -- the live 0.5 MiB
    j = np.arange(OUT_F)
    Wg = weight.reshape(OUT_F, GROUPS, STEP)[j, j % GROUPS]      # [4096, 64]
    W4 = Wg.reshape(M_PER_G, GROUPS, STEP)                       # [m, g, s]
    Wk = W4.reshape(M_PER_G, N_PAIR, 2, STEP)                    # [m, k, h, s]
    # block-diagonal pair stationary: w_bd[64h + s, 64k + 32h' + m]
    w_bd = np.zeros((2, STEP, N_PAIR, 2, M_PER_G), dtype=np.float16)
    for h in range(2):
        w_bd[h, :, :, h, :] = Wk[:, :, h, :].transpose(2, 1, 0)  # [s, k, m]
    w_bd = np.ascontiguousarray(w_bd.reshape(128, N_PAIR * 64))

    # bias in stacked-pair out^T layout: partition p = 64u + 32h + m,
    # tile t -> j = m*128 + 4t + 2u + h
    bias_pp = np.ascontiguousarray(
        bias.reshape(M_PER_G, N_TILE, 2, 2)        # [m, t, u, h]
            .transpose(2, 3, 0, 1)                 # [u, h, m, t]
            .reshape(128, N_TILE)).astype(np.float32)
    return w_bd, bias_pp


def _make_in_maps(x, weight, bias):
    w_bd, bias_pp = _host_prep(weight, bias)
    # xt[c][p, 512k + n] = x[512c + n, 128k + p]
    xt = (x.reshape(N_CORES, B_CORE, N_PAIR, 128)
           .transpose(0, 3, 2, 1)                  # [c, p, k, n]
           .astype(np.float16)
           .reshape(N_CORES, 128, N_PAIR * B_CORE))
    return [{"xt_s": xt[c], "w_bd": w_bd, "bias_pp": bias_pp}
            for c in range(N_CORES)]


def _decode_out(results):
    # y[p, 512t + n] with p = 64u + 32h + m  ->  out[n, m*128 + 4t + 2u + h]
    out = np.empty((BATCH, OUT_F), np.float32)
    for c in range(N_CORES):
        yc = (results[c]["y_s"]
              .reshape(2, 2, M_PER_G, N_TILE, B_CORE)   # [u, h, m, t, n]
              .transpose(4, 2, 3, 0, 1)                 # [n, m, t, u, h]
              .reshape(B_CORE, OUT_F))
        out[c * B_CORE:(c + 1) * B_CORE] = yc
    return out


def kernel(x, weight, bias):
    x = np.asarray(x, dtype=np.float32)
    weight = np.asarray(weight, dtype=np.float32)
    bias = np.asarray(bias, dtype=np.float32)

    nc = _build()
    in_maps = _make_in_maps(x, weight, bias)
    res = run_bass_kernel_spmd(nc, in_maps, core_ids=list(range(N_CORES)))
    return _decode_out(res.results)
